# revision 4
# baseline (speedup 1.0000x reference)
"""GSA layer (Gaussian-biased axial attention) Trainium2 Bass kernel.

Full inputs in, full output out. Shards batch B=8 across 8 NeuronCores
(data parallel, one image per core). Self-contained: hardcodes shapes.

Per-core dataflow (image = 64x64 tokens, D=1024), three phases:
  P0 (projections): stream xT row-chunks (512 tokens = 8 image rows) in
      bf16; Q,K projections (bf16 matmuls, fp32 psum + bias) written to
      SBUF-resident Q,K [128, 8, 4096] bf16 (transposed layout, row-major
      token order) AND staged contiguously to DRAM; V projection (bf16,
      natural [t, e] layout) staged to DRAM.
  P1 (column attention): per 8-column chunk, scores read resident Q
      (small DVE permute to (w, h) order) and resident K via stride-64
      SBUF access patterns -- no DMA gathers. The Gaussian distance bias
      is accumulated into PSUM by a rank-3 fp32 matmul (dist(i,j) =
      i^2 - 2ij + j^2), exact in fp32. V columns read from DRAM natural
      layout (2KB runs). c_out overwrites the dead columns of resident K
      in place (strided copy), leaving K holding c_out row-major.
  P2 (row attention + output): per row-chunk, Q,K chunks re-read from
      DRAM (contiguous bf16), row scores + softmax + AV, add resident
      c_out (contiguous row slice), fused f32r output projection.
Host: transposes/reshapes, folds bv into output bias, unshards output.
"""

import os
import numpy as np
import ml_dtypes

import concourse.bass as bass
import concourse.mybir as mybir
import concourse.tile as tile
from concourse import bacc
from concourse import bass_utils

F32 = mybir.dt.float32
F32R = mybir.dt.float32r
BF16 = mybir.dt.bfloat16
AX = mybir.AxisListType
ALU = mybir.AluOpType
ACTF = mybir.ActivationFunctionType

B, H, W, D = 8, 64, 64, 1024
P = 128
HW = H * W            # 4096 tokens per image
CH = 512              # token chunk (8 image rows / 8 image cols)
NCH = HW // CH        # 8 chunks
EO = D // P           # 8 partition tiles of the 1024 dim

_cache = {}


def _softmax_block(nc, pool, pss, pnT):
    """Softmax over the free axis of the two diagonal [64,64] blocks of the
    PSUM score tile pss [128,128] (bias already accumulated in PSUM), then
    write transposed bf16 weights into pnT [128, 64]."""
    negm = pool.tile([P, 1], F32, tag="sm_negm")
    ssum = pool.tile([P, 1], F32, tag="sm_ssum")
    rinv = pool.tile([P, 1], F32, tag="sm_rinv")
    pn = pool.tile([P, 64], F32, tag="sm_pn")
    pnn = pool.tile([P, 64], BF16, tag="sm_pnn")
    for hl in range(2):
        blk = slice(hl * 64, hl * 64 + 64)
        cblk = slice(hl * 64, hl * 64 + 64)
        nc.vector.tensor_reduce(negm[blk, :], pss[blk, cblk], axis=AX.X,
                                op=ALU.max, negate=True)
        nc.scalar.activation(pn[blk, :], pss[blk, cblk], ACTF.Exp,
                             bias=negm[blk, 0:1], accum_out=ssum[blk, 0:1])
    nc.vector.reciprocal(rinv[:], ssum[:])
    nc.vector.tensor_scalar_mul(pnn[:], pn[:], rinv[:, 0:1])
    # transpose each 64x64 half via 4 DVE 32x32 block transposes
    for hl in range(2):
        o = hl * 64
        for bi in range(2):
            for bj in range(2):
                nc.vector.transpose(
                    pnT[o + bi * 32:o + bi * 32 + 32, bj * 32:bj * 32 + 32],
                    pnn[o + bj * 32:o + bj * 32 + 32, bi * 32:bi * 32 + 32])


def _build():
    nc = bacc.Bacc("TRN2", target_bir_lowering=False, debug=False,
                   num_devices=8)

    xT_d = nc.dram_tensor("xT", [D, HW], BF16, kind="ExternalInput").ap()
    wq_d = nc.dram_tensor("wqT", [D, D], BF16, kind="ExternalInput").ap()
    wk_d = nc.dram_tensor("wkT", [D, D], BF16, kind="ExternalInput").ap()
    wv_d = nc.dram_tensor("wvT", [D, D], BF16, kind="ExternalInput").ap()
    wo_d = nc.dram_tensor("woT", [D, D], F32R, kind="ExternalInput").ap()
    bq_d = nc.dram_tensor("bqt", [P, EO], F32, kind="ExternalInput").ap()
    bk_d = nc.dram_tensor("bkt", [P, EO], F32, kind="ExternalInput").ap()
    bo_d = nc.dram_tensor("bot", [P, EO], F32, kind="ExternalInput").ap()
    # rank-3 factorization of -gw*dist: score_bias[i, j] = sum_r L[r,i]*R[r,j]
    bL_d = nc.dram_tensor("biasL", [4, P], F32, kind="ExternalInput").ap()
    bR_d = nc.dram_tensor("biasR", [4, P], F32, kind="ExternalInput").ap()
    out_d = nc.dram_tensor("outT", [D, HW], F32, kind="ExternalOutput").ap()

    xTv = xT_d.rearrange("(do p) t -> p do t", p=P)      # [128, 8, 4096]
    wqv = wq_d.rearrange("(do p) e -> p do e", p=P)
    wkv = wk_d.rearrange("(do p) e -> p do e", p=P)
    wvv = wv_d.rearrange("(do p) e -> p do e", p=P)
    wov = wo_d.rearrange("(do p) e -> p do e", p=P)
    outv = out_d.rearrange("(eo p) t -> p eo t", p=P)

    with tile.TileContext(nc) as tc:
      with tc.tile_pool(name="dram", bufs=1, space="DRAM") as dpool, \
           tc.tile_pool(name="consts", bufs=1) as cpool:
        qn_d = dpool.tile([P, EO, HW], BF16)      # Q^T row-major [p, eo, t]
        kn_d = dpool.tile([P, EO, HW], BF16)
        vn_d = dpool.tile([HW, D], BF16)          # V natural row-order

        bo_sb = cpool.tile([P, EO], F32)
        nc.sync.dma_start(bo_sb[:], bo_d)
        bL_sb = cpool.tile([4, P], F32)
        nc.sync.dma_start(bL_sb[:], bL_d)
        bR_sb = cpool.tile([4, P], F32)
        nc.sync.dma_start(bR_sb[:], bR_d)

        with tc.tile_pool(name="kres", bufs=1) as kpool:
          kres = kpool.tile([P, EO, HW], BF16)     # K^T resident; c_out later
          kres_v = kres[:].rearrange("p eo (h w) -> p eo h w", w=64)

          with tc.tile_pool(name="qres", bufs=1) as qpool:
            qres = qpool.tile([P, EO, HW], BF16)   # Q^T resident
            qres_v = qres[:].rearrange("p eo (h w) -> p eo h w", w=64)

            # ---------------- P0: projections ----------------
            with tc.tile_pool(name="wA", bufs=1) as wA, \
                 tc.tile_pool(name="pX", bufs=2) as pX, \
                 tc.tile_pool(name="pV", bufs=1) as pV, \
                 tc.tile_pool(name="psProj", bufs=3, space="PSUM") as psProj:
                wq_sb = wA.tile([P, EO, D], BF16)
                wk_sb = wA.tile([P, EO, D], BF16)
                wv_sb = wA.tile([P, EO, D], BF16)
                nc.sync.dma_start(wq_sb[:], wqv)
                nc.sync.dma_start(wk_sb[:], wkv)
                nc.sync.dma_start(wv_sb[:], wvv)
                bq_sb = wA.tile([P, EO], F32)
                bk_sb = wA.tile([P, EO], F32)
                nc.sync.dma_start(bq_sb[:], bq_d)
                nc.sync.dma_start(bk_sb[:], bk_d)

                for c in range(NCH):
                    tsl = slice(c * CH, (c + 1) * CH)
                    x_sb = pX.tile([P, EO, CH], BF16, tag="x")
                    nc.sync.dma_start(x_sb[:], xTv[:, :, tsl])

                    for et in range(EO):
                        esl = slice(et * P, (et + 1) * P)
                        psq = psProj.tile([P, CH], F32, tag="pp")
                        for dt_ in range(EO):
                            nc.tensor.matmul(psq[:], wq_sb[:, dt_, esl],
                                             x_sb[:, dt_, :],
                                             start=(dt_ == 0),
                                             stop=(dt_ == EO - 1))
                        nc.scalar.add(qres[:, et, tsl], psq[:],
                                      add=bq_sb[:, et:et + 1])
                        psk = psProj.tile([P, CH], F32, tag="pp")
                        for dt_ in range(EO):
                            nc.tensor.matmul(psk[:], wk_sb[:, dt_, esl],
                                             x_sb[:, dt_, :],
                                             start=(dt_ == 0),
                                             stop=(dt_ == EO - 1))
                        nc.scalar.add(kres[:, et, tsl], psk[:],
                                      add=bk_sb[:, et:et + 1])

                    # V natural [t, e] bf16
                    v_sb = pV.tile([P, CH // P, D], BF16, tag="v")
                    for tt in range(CH // P):
                        for eh in range(2):
                            psv = psProj.tile([P, 512], F32, tag="pp")
                            for dt_ in range(EO):
                                nc.tensor.matmul(
                                    psv[:], x_sb[:, dt_, tt * P:(tt + 1) * P],
                                    wv_sb[:, dt_, eh * 512:(eh + 1) * 512],
                                    start=(dt_ == 0), stop=(dt_ == EO - 1))
                            nc.scalar.copy(
                                v_sb[:, tt, eh * 512:(eh + 1) * 512], psv[:])
                    nc.sync.dma_start(
                        vn_d[tsl, :].rearrange("(tt p) e -> p tt e", p=P),
                        v_sb[:])
                    # contiguous DRAM staging of Q,K for P2
                    nc.sync.dma_start(qn_d[:, :, tsl], qres[:, :, tsl])
                    nc.sync.dma_start(kn_d[:, :, tsl], kres[:, :, tsl])

            # ---------------- P1: column attention ----------------
            vn_v = vn_d[:].rearrange("(h w) e -> h w e", w=64)
            with tc.tile_pool(name="pQc", bufs=2) as pQc, \
                 tc.tile_pool(name="pVc", bufs=2) as pVc, \
                 tc.tile_pool(name="pSm", bufs=3) as pSm, \
                 tc.tile_pool(name="psSc", bufs=2, space="PSUM") as psSc, \
                 tc.tile_pool(name="psAv", bufs=2, space="PSUM") as psAv:
                for c in range(NCH):
                    wsl = slice(c * 8, (c + 1) * 8)
                    # permute Q cols to (w, h) contiguous order for lhsT
                    qc = pQc.tile([P, EO, 8, 64], BF16, tag="qc")
                    nc.vector.tensor_copy(
                        qc[:], qres_v[:, :, :, wsl].rearrange(
                            "p eo h w -> p eo w h"))
                    qc_f = qc[:].rearrange("p eo w h -> p eo (w h)")
                    vcw = pVc.tile([P, 4, D], BF16, tag="vcw")
                    for wl in range(8):
                        w_abs = c * 8 + wl
                        nc.sync.dma_start(
                            vcw[(wl % 2) * 64:(wl % 2) * 64 + 64, wl // 2, :],
                            vn_v[:, w_abs, :])

                    for pr in range(4):
                        psl = slice(pr * P, (pr + 1) * P)
                        pss = psSc.tile([P, P], F32, tag="sc")
                        # rank-3 fp32 bias matmul fills the whole bank
                        nc.tensor.matmul(pss[:], bL_sb[:], bR_sb[:],
                                         start=True, stop=False)
                        for et in range(EO):
                            lhsT = qc_f[:, et, psl]
                            for wl2 in range(2):
                                w_abs = c * 8 + pr * 2 + wl2
                                nc.tensor.matmul(
                                    pss[:, wl2 * 64:wl2 * 64 + 64], lhsT,
                                    kres_v[:, et, :, w_abs],
                                    start=False, stop=(et == EO - 1),
                                    skip_group_check=True)
                        pnT = pSm.tile([P, 64], BF16, tag="sm_pnT")
                        _softmax_block(nc, pSm, pss, pnT)
                        psav = psAv.tile([P, 1024], F32, tag="av")
                        for wl2 in range(2):
                            w_loc = pr * 2 + wl2
                            vp = (w_loc % 2) * 64
                            wo_ = w_loc // 2
                            for ds_ in range(EO):
                                nc.tensor.matmul(
                                    psav[:, ds_ * P + wl2 * 64:
                                         ds_ * P + wl2 * 64 + 64],
                                    vcw[vp:vp + 64, wo_, ds_ * P:(ds_ + 1) * P],
                                    pnT[wl2 * 64:wl2 * 64 + 64, :],
                                    start=True, stop=True,
                                    skip_group_check=True)
                        # c_out overwrites dead K columns (strided, row-major)
                        w0 = c * 8 + pr * 2
                        nc.scalar.copy(
                            kres_v[:, :, :, w0:w0 + 2].rearrange(
                                "p ds h w -> p ds w h"),
                            psav[:].rearrange("p (ds wl h) -> p ds wl h",
                                              ds=8, wl=2))

          # qres freed here ------------------------------------------------
          # ---------------- P2: row attention + output projection --------
          with tc.tile_pool(name="wB", bufs=1) as wB, \
               tc.tile_pool(name="pQK", bufs=2) as pQK, \
               tc.tile_pool(name="pVr", bufs=2) as pVr, \
               tc.tile_pool(name="pSum", bufs=2) as pSum, \
               tc.tile_pool(name="pOut", bufs=2) as pOut, \
               tc.tile_pool(name="pSmB", bufs=3) as pSm, \
               tc.tile_pool(name="psScB", bufs=2, space="PSUM") as psSc, \
               tc.tile_pool(name="psAvB", bufs=2, space="PSUM") as psAv, \
               tc.tile_pool(name="psPrB", bufs=2, space="PSUM") as psProj:
            wo_sb = wB.tile([P, EO, D], F32R)
            nc.sync.dma_start(wo_sb[:], wov)
            vn_p = vn_d[:].rearrange("(t p) e -> p t e", p=P)   # [128,32,1024]

            for c in range(NCH):
                tsl = slice(c * CH, (c + 1) * CH)
                qch = pQK.tile([P, EO, CH], BF16, tag="qch")
                kch = pQK.tile([P, EO, CH], BF16, tag="kch")
                nc.sync.dma_start(qch[:], qn_d[:, :, tsl])
                nc.sync.dma_start(kch[:], kn_d[:, :, tsl])

                sum_sb = pSum.tile([P, EO, CH], F32R, tag="sum")
                for pr in range(4):
                    psl = slice(pr * P, (pr + 1) * P)
                    vr = pVr.tile([P, D], BF16, tag="vr")
                    nc.sync.dma_start(vr[:], vn_p[:, c * 4 + pr, :])
                    pss = psSc.tile([P, P], F32, tag="sc")
                    nc.tensor.matmul(pss[:], bL_sb[:], bR_sb[:],
                                     start=True, stop=False)
                    for et in range(EO):
                        nc.tensor.matmul(pss[:], qch[:, et, psl],
                                         kch[:, et, psl],
                                         start=False, stop=(et == EO - 1),
                                         skip_group_check=True)
                    pnT = pSm.tile([P, 64], BF16, tag="sm_pnT")
                    _softmax_block(nc, pSm, pss, pnT)
                    psav = psAv.tile([P, 1024], F32, tag="av")
                    for hl in range(2):
                        for ds_ in range(EO):
                            nc.tensor.matmul(
                                psav[:, ds_ * P + hl * 64:
                                     ds_ * P + hl * 64 + 64],
                                vr[hl * 64:hl * 64 + 64, ds_ * P:(ds_ + 1) * P],
                                pnT[hl * 64:hl * 64 + 64, :],
                                start=True, stop=True, skip_group_check=True)
                    # sum = r_out + c_out (c_out lives in kres, row-major)
                    nc.vector.tensor_tensor(
                        sum_sb[:, :, psl].rearrange(
                            "p ds (hl w) -> p ds hl w", hl=2),
                        psav[:].rearrange("p (ds hl w) -> p ds hl w",
                                          ds=8, hl=2),
                        kres[:, :, c * CH + pr * P:c * CH + (pr + 1) * P]
                        .rearrange("p ds (hl w) -> p ds hl w", hl=2),
                        ALU.add)

                outT_sb = pOut.tile([P, EO, CH], F32, tag="out")
                for et in range(EO):
                    esl = slice(et * P, (et + 1) * P)
                    pso = psProj.tile([P, CH], F32, tag="po")
                    for dt_ in range(EO):
                        nc.tensor.matmul(pso[:], wo_sb[:, dt_, esl],
                                         sum_sb[:, dt_, :],
                                         start=(dt_ == 0),
                                         stop=(dt_ == EO - 1))
                    nc.scalar.add(outT_sb[:, et, :], pso[:],
                                  add=bo_sb[:, et:et + 1])
                nc.sync.dma_start(outv[:, :, tsl], outT_sb[:])

    nc.compile()
    return nc


def kernel(x, Wq, bq, Wk, bk, Wv, bv, Wo, bo, sigma, **_ignored):
    x = np.asarray(x, np.float32)
    Wq = np.asarray(Wq, np.float32)
    Wk = np.asarray(Wk, np.float32)
    Wv = np.asarray(Wv, np.float32)
    Wo = np.asarray(Wo, np.float32)
    bq = np.asarray(bq, np.float32)
    bk = np.asarray(bk, np.float32)
    bv = np.asarray(bv, np.float32)
    bo = np.asarray(bo, np.float32)
    sigma = np.asarray(sigma, np.float32)

    if "nc" not in _cache:
        _cache["nc"] = _build()
    nc = _cache["nc"]

    gw = 1.0 / (2.0 * float(sigma[0]) ** 2)
    i = np.arange(P, dtype=np.float32) % 64
    # score_bias[i, j] = -gw*(i-j)^2 = L[:, i] . R[:, j]
    bL = np.stack([i * i, np.ones(P, np.float32), i,
                   np.zeros(P, np.float32)]).astype(np.float32)
    bR = np.stack([-gw * np.ones(P, np.float32), -gw * i * i, 2.0 * gw * i,
                   np.zeros(P, np.float32)]).astype(np.float32)

    bf = ml_dtypes.bfloat16
    wqT = np.ascontiguousarray(Wq.T).astype(bf)
    wkT = np.ascontiguousarray(Wk.T).astype(bf)
    wvT = np.ascontiguousarray(Wv.T).astype(bf)
    woT = np.ascontiguousarray(Wo.T)
    # fold bv: softmax rows sum to 1 -> out += 2 * bv @ Wo^T
    bo_eff = bo + 2.0 * (Wo @ bv)
    bqt = np.ascontiguousarray(bq.reshape(EO, P).T)
    bkt = np.ascontiguousarray(bk.reshape(EO, P).T)
    bot = np.ascontiguousarray(bo_eff.astype(np.float32).reshape(EO, P).T)

    in_maps = []
    for b in range(B):
        xT = np.ascontiguousarray(x[b].reshape(HW, D).T).astype(bf)
        in_maps.append({
            "xT": xT, "wqT": wqT, "wkT": wkT, "wvT": wvT, "woT": woT,
            "bqt": bqt, "bkt": bkt, "bot": bot, "biasL": bL, "biasR": bR,
        })

    trace = bool(int(os.environ.get("GSA_TRACE", "0")))
    ncore = int(os.environ.get("GSA_CORES", str(B)))
    try:
        res = bass_utils.run_bass_kernel_spmd(
            nc, in_maps[:ncore], core_ids=list(range(ncore)),
            trace=trace, trace_cores=[0] if trace else None)
    except ImportError:
        # profiling hook unavailable in this environment; run without trace
        os.environ["BASS_NEVER_TRACE"] = "1"
        res = bass_utils.run_bass_kernel_spmd(
            nc, in_maps[:ncore], core_ids=list(range(ncore)))
    _cache["last_results"] = res

    out = np.zeros((B, H, W, D), dtype=np.float32)
    for b in range(ncore):
        oT = res.results[b]["outT"]                    # [1024, 4096] t=h*64+w
        out[b] = oT.reshape(D, H, W).transpose(1, 2, 0)
    return out


def _kernel_jax_fallback(x, Wq, bq, Wk, bk, Wv, bv, Wo, bo, sigma):
    """Data-parallel jax implementation over the 8 cores (safety net)."""
    import jax
    import jax.numpy as jnp

    def one(xb, Wq, bq, Wk, bk, Wv, bv, Wo, bo, gw, dist):
        q = jnp.einsum("hwd,ed->hwe", xb, Wq) + bq
        k = jnp.einsum("hwd,ed->hwe", xb, Wk) + bk
        v = jnp.einsum("hwd,ed->hwe", xb, Wv) + bv
        r_qk = jnp.einsum("hwd,hkd->hwk", q, k) - gw * dist
        r_out = jnp.einsum("hwk,hkd->hwd", jax.nn.softmax(r_qk, axis=-1), v)
        c_qk = jnp.einsum("hwd,gwd->whg", q, k) - gw * dist
        c_out = jnp.einsum("whg,gwd->hwd", jax.nn.softmax(c_qk, axis=-1), v)
        return jnp.einsum("hwd,ed->hwe", r_out + c_out, Wo) + bo

    gw = 1.0 / (2.0 * jnp.square(sigma[0]))
    i = jnp.arange(64)
    dist = jnp.square(i[:, None] - i[None, :]).astype(jnp.float32)
    f = jax.pmap(one, in_axes=(0, None, None, None, None, None, None, None,
                               None, None, None))
    out = f(x, Wq, bq, Wk, bk, Wv, bv, Wo, bo, gw, dist)
    return np.asarray(out, dtype=np.float32)


_kernel_bass = kernel


def kernel_safe(x, Wq, bq, Wk, bk, Wv, bv, Wo, bo, sigma, **_ignored):
    if os.environ.get("GSA_BASS", "1") == "1":
        try:
            return _kernel_bass(x, Wq, bq, Wk, bk, Wv, bv, Wo, bo, sigma)
        except Exception:
            import traceback
            traceback.print_exc()
            print("bass kernel failed; using jax fallback", flush=True)
    return _kernel_jax_fallback(
            np.asarray(x, np.float32), np.asarray(Wq, np.float32),
            np.asarray(bq, np.float32), np.asarray(Wk, np.float32),
            np.asarray(bk, np.float32), np.asarray(Wv, np.float32),
            np.asarray(bv, np.float32), np.asarray(Wo, np.float32),
            np.asarray(bo, np.float32), np.asarray(sigma, np.float32))


kernel = kernel_safe


# revision 82
# speedup vs baseline: 1.3639x; 1.3639x over previous
"""GSA layer (Gaussian-biased axial attention) Trainium2 Bass kernel.

Full inputs in, full output out. Shards batch B=8 across 8 NeuronCores
(data parallel, one image per core). Self-contained: hardcodes shapes.

Key algebraic trick: scores(a,b) = (x_a Wq^T + bq).(x_b Wk^T + bk)
 = x_a M x_b^T + u.x_b + const_a  with M = Wq^T Wk, u = bq @ Wk, and the
const_a term dropped (softmax-invariant). So a single projection
qt = x @ M + u replaces both Q and K, and raw x plays the key role.
The Gaussian bias -gw*(i-j)^2 is rank-3 (i^2, -2ij, j^2) and accumulated
into PSUM by one small fp32 matmul. Attention path runs in fp16 (1 cyc/row
on the PE like bf16, but 8x more precise -- bf16 fails the 2e-2 gate).

Per-core dataflow (image = 64x64 tokens, D=1024), three phases:
  P0: stream xT row-chunks into SBUF-resident x [128,8,4096] f16; compute
      qt (resident, f16) and V (natural [t,e] f16, staged to DRAM).
  P1 (column attention): scores read resident qt (small DVE permute to
      (w,h) order) and resident x via stride-64 SBUF access patterns --
      no DMA gathers. V columns from DRAM natural layout (2KB runs).
      c_out overwrites the dead columns of resident x in place.
  P2 (row attention + output): row scores read resident qt (contiguous)
      and x chunks re-fetched from the xT input (contiguous); AV from
      V row slices; add resident c_out (contiguous); fused f16 output
      projection with folded bias (bo + 2*Wo@bv).
Host: transposes/reshapes, computes M/u/bias factors, unshards output.
"""

import os
import numpy as np
import ml_dtypes

import concourse.bass as bass
import concourse.mybir as mybir
import concourse.tile as tile
from concourse import bacc
from concourse import bass_utils

F32 = mybir.dt.float32
F16 = mybir.dt.float16
F8 = mybir.dt.float8e4
PERF_DR = mybir.MatmulPerfMode.DoubleRow
AX = mybir.AxisListType
ALU = mybir.AluOpType
ACTF = mybir.ActivationFunctionType

B, H, W, D = 8, 64, 64, 1024
P = 128
HW = H * W            # 4096 tokens per image
CH = 512              # token chunk (8 image rows / 8 image cols)
NCH = HW // CH        # 8 chunks
EO = D // P           # 8 partition tiles of the 1024 dim

_cache = {}
DEBUG_TAPS = bool(int(os.environ.get("GSA_DEBUG_TAPS", "0")))


def _softmax_core(nc, pool, pss):
    """Stages 1 of softmax on the two diagonal [64,64] blocks of pss:
    returns the normalized weights pnn [128, 64] f16 (not yet transposed)."""
    negm = pool.tile([P, 1], F32, tag="sm_negm")
    ssum = pool.tile([P, 1], F32, tag="sm_ssum")
    rinv = pool.tile([P, 1], F32, tag="sm_rinv")
    pn = pool.tile([P, 64], F32, tag="sm_pn")
    pnn = pool.tile([P, 64], F16, tag="sm_pnn")
    for hl in range(2):
        blk = slice(hl * 64, hl * 64 + 64)
        nc.vector.tensor_reduce(negm[blk, :], pss[blk, blk], axis=AX.X,
                                op=ALU.max, negate=True)
        nc.scalar.activation(pn[blk, :], pss[blk, blk], ACTF.Exp,
                             bias=negm[blk, 0:1], accum_out=ssum[blk, 0:1])
    nc.vector.reciprocal(rinv[:], ssum[:])
    nc.gpsimd.tensor_scalar_mul(pnn[:], pn[:], rinv[:, 0:1])
    return pnn


def _softmax_block(nc, pool, pss, pnT):
    """_softmax_core + DVE 32x32 block transposes into pnT [128, 64]."""
    pnn = _softmax_core(nc, pool, pss)
    for hl in range(2):
        o = hl * 64
        for bi in range(2):
            for bj in range(2):
                nc.vector.transpose(
                    pnT[o + bi * 32:o + bi * 32 + 32, bj * 32:bj * 32 + 32],
                    pnn[o + bj * 32:o + bj * 32 + 32, bi * 32:bi * 32 + 32])


def _build():
    nc = bacc.Bacc("TRN2", target_bir_lowering=False, debug=False,
                   num_devices=8)

    xT_d = nc.dram_tensor("xT", [D, HW], F16, kind="ExternalInput").ap()
    xTr_d = nc.dram_tensor("xTr", [D, HW], F16, kind="ExternalInput").ap()
    m_d = nc.dram_tensor("mT", [D, D], F16, kind="ExternalInput").ap()
    wv_d = nc.dram_tensor("wvT", [D, D], F16, kind="ExternalInput").ap()
    wo_d = nc.dram_tensor("woT", [D, D], F16, kind="ExternalInput").ap()
    bu_d = nc.dram_tensor("but", [P, EO], F32, kind="ExternalInput").ap()
    bo_d = nc.dram_tensor("bot", [P, EO], F32, kind="ExternalInput").ap()
    # rank-3 factorization of -gw*dist: score_bias[i, j] = sum_r L[r,i]*R[r,j]
    bL_d = nc.dram_tensor("biasL", [16, P], F16, kind="ExternalInput").ap()
    bR_d = nc.dram_tensor("biasR", [16, P], F16, kind="ExternalInput").ap()
    id_d = nc.dram_tensor("ident", [P, P], F16, kind="ExternalInput").ap()
    out_d = nc.dram_tensor("outT", [D, HW], F32, kind="ExternalOutput").ap()
    if DEBUG_TAPS:
        qt_dbg = nc.dram_tensor("qt_dbg", [P, EO, HW], F16,
                                kind="ExternalOutput").ap()
        co_dbg = nc.dram_tensor("co_dbg", [P, EO, HW], F16,
                                kind="ExternalOutput").ap()
        vn_dbg = nc.dram_tensor("vn_dbg", [HW, D], F16,
                                kind="ExternalOutput").ap()
        sum_dbg = nc.dram_tensor("sum_dbg", [P, EO, HW], F16,
                                 kind="ExternalOutput").ap()

    xTv = xT_d.rearrange("(do p) t -> p do t", p=P)      # [128, 8, 4096] col
    xTrv = xTr_d.rearrange("(do p) t -> p do t", p=P)    # [128, 8, 4096] row
    mv = m_d.rearrange("(do p) e -> p do e", p=P)
    wvv = wv_d.rearrange("(do p) e -> p do e", p=P)
    wov = wo_d.rearrange("(do p) e -> p do e", p=P)
    outv = out_d.rearrange("(eo p) t -> p eo t", p=P)

    with tile.TileContext(nc) as tc:
      with tc.tile_pool(name="dram", bufs=1, space="DRAM") as dpool, \
           tc.tile_pool(name="consts", bufs=1) as cpool:
        vn_d = dpool.tile([HW, D], F16)           # V natural row-order

        bo_sb = cpool.tile([P, EO], F32)
        nc.sync.dma_start(bo_sb[:], bo_d)
        bL_sb = cpool.tile([16, P], F16)
        nc.sync.dma_start(bL_sb[:], bL_d)
        bR_sb = cpool.tile([16, P], F16)
        nc.sync.dma_start(bR_sb[:], bR_d)
        id_sb = cpool.tile([P, P], F16)
        nc.sync.dma_start(id_sb[:], id_d)

        # residents hold tokens in COLUMN-major order t' = w*64 + h, so the
        # column pass (which also WRITES c_out into xres) uses only
        # contiguous slices -- strided access patterns appear only in the
        # read-only row pass, where they cannot create WAR hazards
        with tc.tile_pool(name="xres", bufs=1) as xpool, \
             tc.tile_pool(name="qtres", bufs=1) as qpool:
          xres = xpool.tile([P, EO, HW], F16)      # x^T resident; c_out later
          xres_v = xres[:].rearrange("p eo (w h) -> p eo w h", h=64)
          qtres = qpool.tile([P, EO, HW], F16)     # qt^T resident
          qtres_v = qtres[:].rearrange("p eo (w h) -> p eo w h", h=64)

          # ------- P0+P1 fused: projections + column attention -------
          # P0 chunk c finishes whole image COLUMNS (col-major layout), so
          # the column attention for those columns interleaves right behind
          # it -- its softmax latency hides under the next chunk's
          # projection matmuls, and AV reads V straight from SBUF.
          with tc.tile_pool(name="wA", bufs=1) as wA, \
               tc.tile_pool(name="pV", bufs=3) as pV, \
               tc.tile_pool(name="pSm", bufs=4) as pSm, \
               tc.tile_pool(name="psProj", bufs=2, space="PSUM") as psProj, \
               tc.tile_pool(name="psSc", bufs=2, space="PSUM") as psSc, \
               tc.tile_pool(name="psAv", bufs=2, space="PSUM") as psAv:
            m_sb = wA.tile([P, EO, D], F16)
            wv_sb = wA.tile([P, EO, D], F16)
            nc.sync.dma_start(m_sb[:], mv)
            nc.sync.dma_start(wv_sb[:], wvv)
            bu_sb = wA.tile([P, EO], F32)
            nc.sync.dma_start(bu_sb[:], bu_d)

            NG = NCH * 4
            LAG = 2
            v_l = {}
            pnT_l = {}

            def p1_step(g):
                gl = g - LAG
                if g < NG:
                    # scores for step g
                    gsl = slice(g * P, (g + 1) * P)
                    pss = psSc.tile([P, P], F32, tag="sc")
                    # rank-3 fp32 bias matmul fills the whole bank
                    nc.tensor.matmul(pss[:], bL_sb[:], bR_sb[:],
                                     start=True, stop=False,
                                     skip_group_check=True)
                    for et in range(EO):
                        lhsT = qtres[:, et, gsl]
                        for wl2 in range(2):
                            w_abs = g * 2 + wl2
                            nc.tensor.matmul(
                                pss[:, wl2 * 64:wl2 * 64 + 64], lhsT,
                                xres[:, et, w_abs * 64:(w_abs + 1) * 64],
                                start=False, stop=(et == EO - 1),
                                skip_group_check=True)
                if gl >= 0:
                    # AV + c_out drain for step g-LAG (weights transposed on
                    # the DVE; V comes straight from the chunk's SBUF tile)
                    pnn = pnT_l.pop(gl)
                    pnT = pSm.tile([P, 64], F16, tag="sm_pnT")
                    for hl in range(2):
                        o = hl * 64
                        for bi in range(2):
                            for bj in range(2):
                                nc.vector.transpose(
                                    pnT[o + bi * 32:o + bi * 32 + 32,
                                        bj * 32:bj * 32 + 32],
                                    pnn[o + bj * 32:o + bj * 32 + 32,
                                        bi * 32:bi * 32 + 32])
                    cv = gl // 4
                    v_sb = v_l.pop(cv) if gl % 4 == 3 else v_l[cv]
                    psav = psAv.tile([P, 1024], F32, tag="av")
                    for wl2 in range(2):
                        for ds_ in range(EO):
                            nc.tensor.matmul(
                                psav[:, ds_ * P + wl2 * 64:
                                     ds_ * P + wl2 * 64 + 64],
                                v_sb[wl2 * 64:wl2 * 64 + 64, gl % 4,
                                     ds_ * P:(ds_ + 1) * P],
                                pnT[wl2 * 64:wl2 * 64 + 64, :],
                                start=True, stop=True,
                                skip_group_check=True)
                    # c_out overwrites the dead x columns -- contiguous;
                    # drain each psum BANK with a different engine
                    src = psav[:].rearrange("p (ds t) -> p ds t", ds=8)
                    dst = xres[:, :, gl * P:(gl + 1) * P]
                    engs = [(nc.vector.tensor_copy, nc.scalar.copy),
                            (nc.gpsimd.tensor_copy, nc.vector.tensor_copy),
                            (nc.scalar.copy, nc.gpsimd.tensor_copy),
                            (nc.vector.tensor_copy, nc.scalar.copy)][gl % 4]
                    engs[0](dst[:, 0:4, :], src[:, 0:4, :])
                    engs[1](dst[:, 4:8, :], src[:, 4:8, :])
                if g < NG:
                    # softmax core for step g
                    pnT_l[g] = _softmax_core(nc, pSm, pss)

            for c in range(NCH):
                tsl = slice(c * CH, (c + 1) * CH)
                nc.sync.dma_start(xres[:, :, tsl], xTv[:, :, tsl])

                for et in range(EO):
                    esl = slice(et * P, (et + 1) * P)
                    psq = psProj.tile([P, CH], F32, tag="pp")
                    for dt_ in range(EO):
                        nc.tensor.matmul(psq[:], m_sb[:, dt_, esl],
                                         xres[:, dt_, tsl],
                                         start=(dt_ == 0),
                                         stop=(dt_ == EO - 1))
                    nc.scalar.add(qtres[:, et, tsl], psq[:],
                                  add=bu_sb[:, et:et + 1])

                # V natural [t, e] f16; drains split between Act and GpSimd
                v_sb = pV.tile([P, CH // P, D], F16, tag="v")
                v_l[c] = v_sb
                for tt in range(CH // P):
                    for eh in range(2):
                        psv = psProj.tile([P, 512], F32, tag="pp")
                        for dt_ in range(EO):
                            nc.tensor.matmul(
                                psv[:],
                                xres[:, dt_, c * CH + tt * P:
                                     c * CH + (tt + 1) * P],
                                wv_sb[:, dt_, eh * 512:(eh + 1) * 512],
                                start=(dt_ == 0), stop=(dt_ == EO - 1))
                        if (tt * 2 + eh) % 2 == 0:
                            nc.scalar.copy(
                                v_sb[:, tt, eh * 512:(eh + 1) * 512], psv[:])
                        else:
                            nc.gpsimd.tensor_copy(
                                v_sb[:, tt, eh * 512:(eh + 1) * 512], psv[:])
                # store via the Act DMA queue so its input-wait cannot block
                # the next chunk's x load on the SP queue
                nc.scalar.dma_start(
                    vn_d[tsl, :].rearrange("(tt p) e -> p tt e", p=P),
                    v_sb[:])
                # interleave the PREVIOUS chunk's column attention here: all
                # of its inputs are a full chunk old, so nothing stalls
                if c >= 1:
                    for i in range(4):
                        p1_step((c - 1) * 4 + i)
            for g in range(NG - 4, NG + LAG):
                p1_step(g)

          if DEBUG_TAPS:
              nc.sync.dma_start(qt_dbg, qtres[:])
              nc.sync.dma_start(co_dbg, xres[:])
              nc.sync.dma_start(vn_dbg, vn_d[:])

          # ---------------- P2: row attention + output projection --------
          with tc.tile_pool(name="wB", bufs=1) as wB, \
               tc.tile_pool(name="pXc", bufs=2) as pXc, \
               tc.tile_pool(name="pVr", bufs=3) as pVr, \
               tc.tile_pool(name="pSum", bufs=2) as pSum, \
               tc.tile_pool(name="pOut", bufs=1) as pOut, \
               tc.tile_pool(name="pSmB", bufs=4) as pSm, \
               tc.tile_pool(name="psScB", bufs=2, space="PSUM") as psSc, \
               tc.tile_pool(name="psAvB", bufs=2, space="PSUM") as psAv, \
               tc.tile_pool(name="psPrB", bufs=2, space="PSUM") as psProj:
            wo_sb = wB.tile([P, EO, D], F16)
            nc.sync.dma_start(wo_sb[:], wov)
            vn_r = vn_d[:].rearrange("(w h) e -> h w e", h=64)  # row view

            # software-pipelined like P1: scores(g) issue LAG steps ahead
            NG = NCH * 4
            LAG = 2
            xch_l = {}
            vr_l = {}
            pnT_l = {}
            sum_l = {}
            for g in range(NG + LAG):
                if g < NG:
                    c, pr = divmod(g, 4)
                    if pr == 0:
                        xch = pXc.tile([P, EO, CH], F16, tag="xch")
                        nc.sync.dma_start(
                            xch[:], xTrv[:, :, c * CH:(c + 1) * CH])
                        xch_l[c] = xch
                        sum_sb = pSum.tile([P, EO, CH], F16, tag="sum")
                        sum_l[c] = sum_sb
                    # V rows from the col-major layout: 128 descriptors of
                    # one full 2KB token vector each -- full DMA speed
                    vr = pVr.tile([P, D], F16, tag="vr")
                    for hl in range(2):
                        nc.sync.dma_start(vr[hl * 64:(hl + 1) * 64, :],
                                          vn_r[2 * g + hl, :, :])
                    vr_l[g] = vr
                    pss = psSc.tile([P, P], F32, tag="sc")
                    nc.tensor.matmul(pss[:], bL_sb[:], bR_sb[:],
                                     start=True, stop=False,
                                     skip_group_check=True)
                    psl = slice(pr * P, (pr + 1) * P)
                    for et in range(EO):
                        # strided row slice of the col-major resident qt
                        lhsT = qtres_v[:, et, :, 2 * g:2 * g + 2].rearrange(
                            "p w h -> p h w")
                        nc.tensor.matmul(pss[:], lhsT,
                                         xch_l[c][:, et, psl],
                                         start=False, stop=(et == EO - 1),
                                         skip_group_check=True)
                    pnT = pSm.tile([P, 64], F16, tag="sm_pnT")
                    _softmax_block(nc, pSm, pss, pnT)
                    pnT_l[g] = pnT
                if g >= LAG:
                    gl = g - LAG
                    c, pr = divmod(gl, 4)
                    pnT = pnT_l.pop(gl)
                    vr = vr_l.pop(gl)
                    psl = slice(pr * P, (pr + 1) * P)
                    psav = psAv.tile([P, 1024], F32, tag="av")
                    # the first AV matmul into EACH psum bank carries
                    # start=True (clears that bank); later slices overwrite
                    # their own pending bytes, so the identity-adds below
                    # accumulate instead of clobbering
                    for hl in range(2):
                        for ds_ in range(EO):
                            nc.tensor.matmul(
                                psav[:, ds_ * P + hl * 64:
                                     ds_ * P + hl * 64 + 64],
                                vr[hl * 64:hl * 64 + 64, ds_ * P:(ds_ + 1) * P],
                                pnT[hl * 64:hl * 64 + 64, :],
                                start=(hl == 0 and ds_ % 4 == 0), stop=False,
                                skip_group_check=True)
                    # accumulate c_out (resident in xres) into psav on the PE
                    for ds_ in range(EO):
                        nc.tensor.matmul(
                            psav[:, ds_ * P:(ds_ + 1) * P], id_sb[:],
                            xres_v[:, ds_, :, 2 * gl:2 * gl + 2].rearrange(
                                "p w h -> p h w"),
                            start=False, stop=(ds_ == EO - 1),
                            skip_group_check=True)
                    # drain sum = r_out + c_out to SBUF (Act/GpSimd alternate)
                    dst = sum_l[c][:, :, psl].rearrange(
                        "p ds (hl w) -> p ds hl w", hl=2)
                    src = psav[:].rearrange("p (ds hl w) -> p ds hl w",
                                            ds=8, hl=2)
                    if pr % 2 == 0:
                        nc.scalar.copy(dst, src)
                    else:
                        nc.gpsimd.tensor_copy(dst, src)
                    if pr == 3:
                        # chunk complete: fused output projection
                        tsl = slice(c * CH, (c + 1) * CH)
                        sum_sb = sum_l.pop(c)
                        if DEBUG_TAPS:
                            nc.sync.dma_start(sum_dbg[:, :, tsl], sum_sb[:])
                        outT_sb = pOut.tile([P, EO, CH], F32, tag="out")
                        for et in range(EO):
                            esl = slice(et * P, (et + 1) * P)
                            pso = psProj.tile([P, CH], F32, tag="po")
                            for dt_ in range(EO):
                                nc.tensor.matmul(pso[:], wo_sb[:, dt_, esl],
                                                 sum_sb[:, dt_, :],
                                                 start=(dt_ == 0),
                                                 stop=(dt_ == EO - 1))
                            nc.vector.tensor_scalar_add(
                                outT_sb[:, et, :], pso[:],
                                bo_sb[:, et:et + 1])
                            # per-et store issued right after its own add,
                            # overlapping the remaining projections
                            nc.scalar.dma_start(outv[:, et, tsl],
                                                outT_sb[:, et, :])

    nc.compile()
    return nc


def kernel(x, Wq, bq, Wk, bk, Wv, bv, Wo, bo, sigma, **_ignored):
    x = np.asarray(x, np.float32)
    Wq = np.asarray(Wq, np.float32)
    Wk = np.asarray(Wk, np.float32)
    Wv = np.asarray(Wv, np.float32)
    Wo = np.asarray(Wo, np.float32)
    bq = np.asarray(bq, np.float32)
    bk = np.asarray(bk, np.float32)
    bv = np.asarray(bv, np.float32)
    bo = np.asarray(bo, np.float32)
    sigma = np.asarray(sigma, np.float32)

    if "nc" not in _cache:
        _cache["nc"] = _build()
    nc = _cache["nc"]

    gw = 1.0 / (2.0 * float(sigma[0]) ** 2)
    i = np.arange(P, dtype=np.float64) % 64
    # score_bias[i, j] = -gw*(i-j)^2 = L[:, i] . R[:, j], factored into an
    # exact f16 hi/lo split so the bias matmul runs at f16 speed
    L4 = np.stack([i * i, np.ones(P), i, np.zeros(P)])
    R4 = np.stack([-gw * np.ones(P), -gw * i * i, 2.0 * gw * i, np.zeros(P)])
    f16 = np.float16
    Lh = L4.astype(f16).astype(np.float64)
    Ll = (L4 - Lh).astype(f16).astype(np.float64)
    Rh = R4.astype(f16).astype(np.float64)
    Rl = (R4 - Rh).astype(f16).astype(np.float64)
    bL = np.ascontiguousarray(np.concatenate([Lh, Lh, Ll, Ll])).astype(f16)
    bR = np.ascontiguousarray(np.concatenate([Rh, Rl, Rh, Rl])).astype(f16)

    M = (Wq.T @ Wk).astype(np.float32)       # scores = x M x^T (+ u.x_b)
    u = bq @ Wk                              # k-dependent bias term
    mT = np.ascontiguousarray(M).astype(f16)  # [d, d'] laid out like wT
    wvT = np.ascontiguousarray(Wv.T).astype(f16)
    woT = np.ascontiguousarray(Wo.T).astype(f16)
    # fold bv: softmax rows sum to 1 -> out += 2 * bv @ Wo^T
    bo_eff = bo + 2.0 * (Wo @ bv)
    but = np.ascontiguousarray(u.reshape(EO, P).T).astype(np.float32)
    bot = np.ascontiguousarray(bo_eff.astype(np.float32).reshape(EO, P).T)

    in_maps = []
    for b in range(B):
        xTr = np.ascontiguousarray(x[b].reshape(HW, D).T).astype(f16)
        xcol = np.ascontiguousarray(x[b].transpose(1, 0, 2).reshape(HW, D).T)
        in_maps.append({
            "xT": xcol.astype(f16), "xTr": xTr,
            "mT": mT, "wvT": wvT, "woT": woT,
            "but": but, "bot": bot, "biasL": bL, "biasR": bR,
            "ident": np.eye(P, dtype=f16),
        })

    trace = bool(int(os.environ.get("GSA_TRACE", "0")))
    ncore = int(os.environ.get("GSA_CORES", str(B)))
    try:
        res = bass_utils.run_bass_kernel_spmd(
            nc, in_maps[:ncore], core_ids=list(range(ncore)),
            trace=trace, trace_cores=[0] if trace else None)
    except ImportError:
        # profiling hook unavailable in this environment; run without trace
        os.environ["BASS_NEVER_TRACE"] = "1"
        res = bass_utils.run_bass_kernel_spmd(
            nc, in_maps[:ncore], core_ids=list(range(ncore)))
    _cache["last_results"] = res

    out = np.zeros((B, H, W, D), dtype=np.float32)
    for b in range(ncore):
        oT = res.results[b]["outT"]                    # [1024, 4096] t=h*64+w
        out[b] = oT.reshape(D, H, W).transpose(1, 2, 0)
    return out


def _kernel_jax_fallback(x, Wq, bq, Wk, bk, Wv, bv, Wo, bo, sigma):
    """Data-parallel jax implementation over the 8 cores (safety net)."""
    import jax
    import jax.numpy as jnp

    def one(xb, Wq, bq, Wk, bk, Wv, bv, Wo, bo, gw, dist):
        q = jnp.einsum("hwd,ed->hwe", xb, Wq) + bq
        k = jnp.einsum("hwd,ed->hwe", xb, Wk) + bk
        v = jnp.einsum("hwd,ed->hwe", xb, Wv) + bv
        r_qk = jnp.einsum("hwd,hkd->hwk", q, k) - gw * dist
        r_out = jnp.einsum("hwk,hkd->hwd", jax.nn.softmax(r_qk, axis=-1), v)
        c_qk = jnp.einsum("hwd,gwd->whg", q, k) - gw * dist
        c_out = jnp.einsum("whg,gwd->hwd", jax.nn.softmax(c_qk, axis=-1), v)
        return jnp.einsum("hwd,ed->hwe", r_out + c_out, Wo) + bo

    gw = 1.0 / (2.0 * jnp.square(sigma[0]))
    i = jnp.arange(64)
    dist = jnp.square(i[:, None] - i[None, :]).astype(jnp.float32)
    f = jax.pmap(one, in_axes=(0, None, None, None, None, None, None, None,
                               None, None, None))
    out = f(x, Wq, bq, Wk, bk, Wv, bv, Wo, bo, gw, dist)
    return np.asarray(out, dtype=np.float32)


_kernel_bass = kernel


def kernel_safe(x, Wq, bq, Wk, bk, Wv, bv, Wo, bo, sigma, **_ignored):
    if os.environ.get("GSA_BASS", "1") == "1":
        try:
            return _kernel_bass(x, Wq, bq, Wk, bk, Wv, bv, Wo, bo, sigma)
        except Exception:
            import traceback
            traceback.print_exc()
            print("bass kernel failed; using jax fallback", flush=True)
    return _kernel_jax_fallback(
            np.asarray(x, np.float32), np.asarray(Wq, np.float32),
            np.asarray(bq, np.float32), np.asarray(Wk, np.float32),
            np.asarray(bk, np.float32), np.asarray(Wv, np.float32),
            np.asarray(bv, np.float32), np.asarray(Wo, np.float32),
            np.asarray(bo, np.float32), np.asarray(sigma, np.float32))


kernel = kernel_safe
